# revision 1
# baseline (speedup 1.0000x reference)
"""Trainium2 Bass kernel for nn_BCA_17274358465235.

Module: out = x + conv1x1_up( softmax(fx @ fy_up^T) @ fself ) with
fx/fself = 2-layer 1x1-conv projections of x, fy = projection of
bilinearly-upsampled y.  B=4, CX=256, CY=512, CM=64, H=W=64 (N=4096
tokens), HY=WY=32.

Sharding: 8 cores = batch(4) x query-row-half(2).  Each core holds all
4096 keys (fy/fself replicated per batch) and 2048 query rows.  No
collectives.

Per-core algorithm (layouts chosen so no transposes are needed):
  fself^T[key, c]  via second projection layer emitted transposed
  sim^T[key, row] = fy_f[:, keys]^T @ fx[:, rows]   (fp32r matmuls,
      two key-chunks packed into PE row-groups 0-1 / 2-3)
  exp on ACT (no max-subtraction: |sim| < 70, fp32-safe)
  fout^T[c, row] += fself^T_chunk^T @ exp_chunk   (PSUM accumulation,
      ones-column in fself^T produces the softmax denominator Z free)
  out = x + W_up @ (fout^T * (1/Z)) + b_up   (b_up via ones-row in W_up)
The y-upsample runs after the channel projection (linear ops commute):
bilinear 2x with half-pixel centers == 0.25/0.75 stencil with edge
clamping, on DVE.
"""
import sys

for _p in ("/opt/pypackages", "/opt/trn_rl_repo"):
    if _p not in sys.path:
        sys.path.insert(0, _p)

import numpy as np

import concourse.bacc as bacc
import concourse.mybir as mybir
import concourse.tile as tile
from concourse.bass_utils import run_bass_kernel_spmd

F32 = mybir.dt.float32
F32R = mybir.dt.float32r
BF16 = mybir.dt.bfloat16
EXP = mybir.ActivationFunctionType.Exp
COPY = mybir.ActivationFunctionType.Copy

B, CX, CY, CM = 4, 256, 512, 64
H = W = 64
HY = WY = 32
N = H * W              # 4096 tokens
NH = N // 2            # 2048 query rows per core
NYC = HY * WY          # 1024 coarse tokens
KC = N // 128          # 32 key chunks
NU = 2 * KC            # 64 pipeline units (key chunk x row half)

_CACHE = {}


def _build(debug=False):
    nc = bacc.Bacc("TRN2", target_bir_lowering=False, debug=False,
                   enable_asserts=False)

    # ---- DRAM I/O (per-core layouts pre-arranged on host) ----
    # xs: [128, 8 * 1024] block-major: block b = [ch0-127 | ch128-255] of
    #     pixel columns b*512..(b+1)*512  (for fself over the full image)
    xs = nc.dram_tensor("xs", [128, 8192], F32R, kind="ExternalInput").ap()
    # xl: [128, 2 * 2048] ch-chunk-major: this core's 2048 query pixels
    xl = nc.dram_tensor("xl", [128, 4096], F32R, kind="ExternalInput").ap()
    # yb: [128, 4 * 1024] ch-chunk-major
    yb = nc.dram_tensor("yb", [128, 4096], F32R, kind="ExternalInput").ap()
    wpack = nc.dram_tensor("wpack", [128, 1093], F32R, kind="ExternalInput").ap()
    ones = nc.dram_tensor("ones", [1, 4096], F32R, kind="ExternalInput").ap()
    # out: [128, 2 * 2048] ch-chunk-major
    out = nc.dram_tensor("out", [128, 4096], F32, kind="ExternalOutput").ap()
    if debug:
        d_fy2 = nc.dram_tensor("d_fy2", [128, 4096], F32, kind="ExternalOutput").ap()
        d_fx2 = nc.dram_tensor("d_fx2", [128, 2048], F32, kind="ExternalOutput").ap()
        d_fself = nc.dram_tensor("d_fself", [128, 2080], F32, kind="ExternalOutput").ap()
        d_h1s = nc.dram_tensor("d_h1s", [65, 4096], F32, kind="ExternalOutput").ap()
        d_sim0 = nc.dram_tensor("d_sim0", [128, 1024], F32, kind="ExternalOutput").ap()
        d_fout = nc.dram_tensor("d_fout", [65, 2048], F32, kind="ExternalOutput").ap()
        d_scaled = nc.dram_tensor("d_scaled", [65, 2048], F32, kind="ExternalOutput").ap()

    with tile.TileContext(nc) as tc:
        with tc.tile_pool(name="sbW", bufs=1) as sbW, \
             tc.tile_pool(name="sbM", bufs=1) as sbM:
            # ---- long-lived SBUF ----
            t_xl = sbM.tile([128, 4096], F32R)     # fx input + residual
            fy2 = sbM.tile([128, 4096], F32R)      # upsampled fy, duplicated
            fx2 = sbM.tile([128, 2048], F32R)      # fx, duplicated
            fselfT = sbM.tile([128, 65 * KC], BF16)
            h1s_aug = sbM.tile([65, 4096], F32R)   # W_self1 @ x with ones row
            scaled = sbM.tile([65, 2048], F32R)    # [Z/Z; fout/Z] per row

            # ---- weights (single packed blob) ----
            t_wpack = sbW.tile([128, 1093], F32R)
            t_ws1t = t_wpack[:, 0:128]
            t_ws2a = t_wpack[0:65, 128:194]
            t_wx1t = t_wpack[:, 194:322]
            t_wx2t = t_wpack[0:64, 322:386]
            t_bx2 = t_wpack[0:64, 386:387].bitcast(F32)
            t_wy1t = t_wpack[:, 387:643]
            t_wy2t = t_wpack[0:64, 643:707]
            t_by2a = t_wpack[0:64, 707:709].bitcast(F32)
            t_wupt = t_wpack[0:65, 709:965]
            onecol = t_wpack[0:1, 965:1093].bitcast(F32)

            # ================= phase 1: projections =================
            sbP1_cm = tc.tile_pool(name="sbP1", bufs=1)
            sbP1 = sbP1_cm.__enter__()
            with tc.tile_pool(name="sbP2", bufs=1) as sbP2, \
                 tc.tile_pool(name="psP1", bufs=1, space="PSUM") as psP1:
                # input DMAs, critical-path first: weights, then y, then x
                nc.sync.dma_start(t_wpack[:], wpack[:])
                t_yb = sbP2.tile([128, 4096], F32R)
                nc.sync.dma_start(t_yb[:, 0:2048], yb[:, 0:2048])
                nc.sync.dma_start(t_yb[:, 2048:4096], yb[:, 2048:4096])
                nc.sync.dma_start(t_xl[:, 0:2048], xl[:, 0:2048])
                nc.sync.dma_start(t_xl[:, 2048:4096], xl[:, 2048:4096])
                nc.sync.dma_start(h1s_aug[64:65, :], ones[:, 0:4096])
                xs_tiles = []
                for blk in range(8):
                    t_xs = sbP1.tile([128, 1024], F32R, tag="xs", bufs=6,
                                     name=f"t_xs_{blk}")
                    xs_tiles.append(t_xs)
                    nc.sync.dma_start(t_xs[:], xs[:, blk * 1024:(blk + 1) * 1024])

                # warm the ACT exp table early
                t_dum = sbP1.tile([1, 32], F32)
                nc.vector.memset(t_dum[:], 0.0)
                t_dum2 = sbP1.tile([1, 32], F32)
                nc.scalar.activation(t_dum2[:], t_dum[:], EXP)

                # ---- fy path: h1y = Wy1 @ y ; then 0.75/0.25-scaled
                # biased copies (fyc75/fyc25), banded upsample ----
                h1y_s = sbP2.tile([64, 1024], F32R)
                fyc75 = sbP2.tile([64, 1024], F32)
                fyc25 = sbP2.tile([64, 1024], F32)
                for blk in range(2):
                    p = psP1.tile([64, 512], F32, tag="blk", bufs=4,
                                  name=f"p_h1y_{blk}")
                    for a in range(4):
                        nc.tensor.matmul(
                            p[:], t_wy1t[:, a * 64:(a + 1) * 64],
                            t_yb[:, blk * 2048 + a * 512:blk * 2048 + a * 512 + 512],
                            start=(a == 0), stop=(a == 3))
                    nc.scalar.activation(h1y_s[:, blk * 512:blk * 512 + 512],
                                         p[:], COPY)
                MUL, ADD_ = mybir.AluOpType.mult, mybir.AluOpType.add
                for blk in range(2):
                    p = psP1.tile([64, 512], F32, tag="blk", bufs=4,
                                  name=f"p_fyc_{blk}")
                    nc.tensor.matmul(p[:], t_wy2t,
                                     h1y_s[:, blk * 512:blk * 512 + 512],
                                     start=True, stop=True)
                    bs = slice(blk * 512, blk * 512 + 512)
                    nc.vector.tensor_scalar(fyc75[:, bs], p[:], 0.75,
                                            t_by2a[:, 0:1], MUL, ADD_)
                    nc.vector.tensor_scalar(fyc25[:, bs], p[:], 0.25,
                                            t_by2a[:, 1:2], MUL, ADD_)

                # H pass, 2 bands: [64, (32,32)] -> [64, (64,32)]
                fyH = sbP2.tile([64, 2048], F32)
                t1v = fyc75[:].rearrange("p (h w) -> p h w", h=32)
                t2v = fyc25[:].rearrange("p (h w) -> p h w", h=32)
                fe = fyH[:].rearrange("p (h two w) -> p h two w", h=32, two=2)
                # band 0 (needs fyc block 0 only: h 0..16 -> h' 0..30)
                nc.vector.tensor_add(fe[:, 0, 0, :], t1v[:, 0, :], t2v[:, 0, :])
                nc.vector.tensor_add(fe[:, 1:16, 0, :], t1v[:, 1:16, :], t2v[:, 0:15, :])
                nc.vector.tensor_add(fe[:, 0:15, 1, :], t1v[:, 0:15, :], t2v[:, 1:16, :])
                # band 1 (h' 31..63)
                nc.vector.tensor_add(fe[:, 16:32, 0, :], t1v[:, 16:32, :], t2v[:, 15:31, :])
                nc.vector.tensor_add(fe[:, 15:31, 1, :], t1v[:, 15:31, :], t2v[:, 16:32, :])
                nc.vector.tensor_add(fe[:, 31, 1, :], t1v[:, 31, :], t2v[:, 31, :])

                # scaled fyH copies, 2 bands: rows 0..30 / 31..63
                u1 = sbP2.tile([64, 2048], F32, tag="ut", bufs=2, name="u1")
                u2 = sbP2.tile([64, 2048], F32, tag="ut", bufs=2, name="u2")
                u1v = u1[:].rearrange("p (h w) -> p h w", h=64)
                u2v = u2[:].rearrange("p (h w) -> p h w", h=64)
                fyHv = fyH[:].rearrange("p (h w) -> p h w", h=64)
                nc.scalar.activation(u1[:, 0:31 * 32], fyH[:, 0:31 * 32],
                                     COPY, scale=0.75)
                nc.scalar.activation(u2[:, 0:31 * 32], fyH[:, 0:31 * 32],
                                     COPY, scale=0.25)
                nc.scalar.activation(u1[:, 31 * 32:2048], fyH[:, 31 * 32:2048],
                                     COPY, scale=0.75)
                nc.scalar.activation(u2[:, 31 * 32:2048], fyH[:, 31 * 32:2048],
                                     COPY, scale=0.25)

                # W pass + row-group duplication, 4 bands
                fw = fy2[0:64, :].rearrange("p (h w two) -> p h w two", h=64, two=2)
                for hs, c0, c1 in ((slice(0, 16), 0, 1024),
                                   (slice(16, 31), 1024, 1984),
                                   (slice(31, 48), 1984, 3072),
                                   (slice(48, 64), 3072, 4096)):
                    nc.vector.tensor_copy(fw[:, hs, 0, 0], fyHv[:, hs, 0])
                    nc.vector.tensor_add(fw[:, hs, 1:32, 0], u1v[:, hs, 1:32],
                                         u2v[:, hs, 0:31])
                    nc.vector.tensor_add(fw[:, hs, 0:31, 1], u1v[:, hs, 0:31],
                                         u2v[:, hs, 1:32])
                    nc.vector.tensor_copy(fw[:, hs, 31, 1], fyHv[:, hs, 31])
                    nc.vector.tensor_copy(fy2[64:128, c0:c1], fy2[0:64, c0:c1])

                # ---- fx path: h1x = Wx1 @ xl ; fx = Wx2 @ h1x + bx2 ----
                h1x_s = sbP1.tile([64, 2048], F32R)
                for blk in range(4):
                    p = psP1.tile([64, 512], F32, tag="blk", bufs=4,
                                  name=f"p_h1x_{blk}")
                    for a in range(2):
                        nc.tensor.matmul(
                            p[:], t_wx1t[:, a * 64:(a + 1) * 64],
                            t_xl[:, blk * 1024 + a * 512:blk * 1024 + a * 512 + 512],
                            start=(a == 0), stop=(a == 1))
                    nc.scalar.activation(h1x_s[:, blk * 512:blk * 512 + 512],
                                         p[:], COPY)
                for blk in range(4):
                    p = psP1.tile([64, 512], F32, tag="blk", bufs=4,
                                  name=f"p_fx_{blk}")
                    nc.tensor.matmul(p[:], t_wx2t,
                                     h1x_s[:, blk * 512:blk * 512 + 512],
                                     start=True, stop=True)
                    nc.vector.tensor_scalar_add(fx2[0:64, blk * 512:blk * 512 + 512],
                                                p[:], t_bx2)
                nc.vector.tensor_copy(fx2[64:128, :], fx2[0:64, :])

                # preamble of the fself stream (blocks/chunks 0-1) while the
                # attention pools are not yet open
                for blk in range(2):
                    p = psP1.tile([64, 512], F32, tag="blk", bufs=4,
                                  name=f"pp_h1s_{blk}")
                    for a in range(2):
                        nc.tensor.matmul(p[:], t_ws1t[:, a * 64:(a + 1) * 64],
                                         xs_tiles[blk][:, a * 512:(a + 1) * 512],
                                         start=(a == 0), stop=(a == 1))
                    nc.vector.tensor_copy(
                        h1s_aug[0:64, blk * 512:blk * 512 + 512], p[:])
                for ck in range(2):
                    p2 = psP1.tile([128, 66], F32, tag="blk", bufs=4,
                                   name=f"pp_fs_{ck}")
                    nc.tensor.matmul(p2[:], h1s_aug[:, ck * 128:(ck + 1) * 128],
                                     t_ws2a, start=True, stop=True)
                    nc.vector.tensor_copy(fselfT[:, ck * 65:(ck + 1) * 65],
                                          p2[:, 0:65])

                # ---- fself path: h1s = Ws1 @ xs ; fselfT chunks ----



            # ================= phase 2: attention (two half-loops) ====
            # half-loop h: all 32 key chunks for row half h (1024 rows).
            # fout accumulator per half = 2 PSUM banks, so half-loop 0 can
            # run a dedicated fself/h1s PSUM pool (2 banks) alongside the
            # sim pool (4 banks); half-loop 1 runs with both fout halves
            # live; the 1/Z half-0 tail overlaps half-loop 1 (GPSIMD
            # broadcast needs no PSUM).
            et_tiles = {}
            invzs = {}
            fout_accs = {}

            def sim_unit(pool, ck, h):
                ps = pool.tile([128, 1024], F32, tag="sim", bufs=2,
                               name=f"sim_{ck}_{h}")
                nc.tensor.matmul(
                    ps[:, 0:512], fy2[0:64, ck * 128:(ck + 1) * 128],
                    fx2[0:64, h * 1024:h * 1024 + 512],
                    start=True, stop=True)
                nc.tensor.matmul(
                    ps[:, 512:1024], fy2[64:128, ck * 128:(ck + 1) * 128],
                    fx2[64:128, h * 1024 + 512:h * 1024 + 1024],
                    start=True, stop=True)
                return ps

            def exp_unit(st, ck, h):
                et = sbM.tile([128, 1024], BF16, tag="et",
                              bufs=3 if debug else 4, name=f"et_{ck}_{h}")
                if debug and ck == 0 and h == 0:
                    d0 = sbM.tile([128, 1024], F32)
                    nc.vector.tensor_copy(d0[:], st[:])
                    nc.sync.dma_start(d_sim0[:], d0[:])
                nc.scalar.activation(et[:], st[:], EXP)
                return et

            def pv_unit(fout_acc, et, ck):
                w = fselfT[:, ck * 65:(ck + 1) * 65]
                nc.tensor.matmul(fout_acc[:, 0:512], w, et[:, 0:512],
                                 start=(ck == 0), stop=(ck == KC - 1))
                nc.tensor.matmul(fout_acc[:, 512:1024], w, et[:, 512:1024],
                                 start=(ck == 0), stop=(ck == KC - 1))

            def half_loop(h, psB, fs_hook, preamble=False):
                fout_acc = fout_accs[h]
                sims = {}
                sims[0] = sim_unit(psB, 0, h)
                sims[1] = sim_unit(psB, 1, h)
                sims[2] = sim_unit(psB, 2, h)
                if preamble:
                    fs_hook(-1)
                for ck in range(KC):
                    if fs_hook is not None:
                        fs_hook(ck)
                    et = exp_unit(sims.pop(ck), ck, h)
                    pv_unit(fout_acc, et, ck)
                    if ck + 3 < KC:
                        sims[ck + 3] = sim_unit(psB, ck + 3, h)

            def pre_tail(h):
                # 1/Z and fout scaling; no PSUM needed beyond fout_acc
                fout_acc = fout_accs[h]
                for s in range(2):
                    cs = slice(s * 512, (s + 1) * 512)
                    invz = sbM.tile([1, 512], F32, tag="zrow", bufs=2,
                                    name=f"invz_{h}_{s}")
                    nc.vector.reciprocal_approx_fast(invz[:], fout_acc[0:1, cs])
                    invzb = sbM.tile([128, 512], F32, tag="izb", bufs=2,
                                     name=f"invzb_{h}_{s}")
                    nc.gpsimd.partition_broadcast(invzb[:], invz[:])
                    nc.vector.tensor_mul(
                        scaled[:, h * 1024 + s * 512:h * 1024 + (s + 1) * 512],
                        fout_acc[:, cs], invzb[0:65, :])

            with tc.tile_pool(name="psA0", bufs=1, space="PSUM") as psA0:
                fout_accs[0] = psA0.tile([65, 1024], F32, name="fout0")
                with tc.tile_pool(name="psFS", bufs=1, space="PSUM") as psFS:

                    def fs_mms(ck):
                        p = psFS.tile([128, 66], F32, tag="fs", bufs=2,
                                      name=f"p_fs_{ck}")
                        nc.tensor.matmul(p[:],
                                         h1s_aug[:, ck * 128:(ck + 1) * 128],
                                         t_ws2a, start=True, stop=True)
                        nc.vector.tensor_copy(fselfT[:, ck * 65:(ck + 1) * 65],
                                              p[:, 0:65])

                    def h1s_mms(blk):
                        t_xs = xs_tiles[blk]
                        p = psFS.tile([64, 512], F32, tag="fs", bufs=2,
                                      name=f"p_h1s_{blk}")
                        for a in range(2):
                            nc.tensor.matmul(p[:],
                                             t_ws1t[:, a * 64:(a + 1) * 64],
                                             t_xs[:, a * 512:(a + 1) * 512],
                                             start=(a == 0), stop=(a == 1))
                        nc.vector.tensor_copy(
                            h1s_aug[0:64, blk * 512:blk * 512 + 512], p[:])

                    def fs_hook(ck):
                        if ck == -1:
                            return
                        if 2 * ck + 2 < 8:
                            h1s_mms(2 * ck + 2)
                            h1s_mms(2 * ck + 3)
                        if 2 * ck + 2 < KC:
                            fs_mms(2 * ck + 2)
                            fs_mms(2 * ck + 3)

                    with tc.tile_pool(name="psB0", bufs=1,
                                      space="PSUM") as psB0:
                        half_loop(0, psB0, fs_hook, preamble=True)

                with tc.tile_pool(name="psA1", bufs=1, space="PSUM") as psA1:
                    fout_accs[1] = psA1.tile([65, 1024], F32, name="fout1")
                    with tc.tile_pool(name="psB1", bufs=1,
                                      space="PSUM") as psB1:
                        pre_tail(0)
                        half_loop(1, psB1, None)
                    pre_tail(1)

                    if debug:
                        nc.sync.dma_start(d_fout[:, 0:1024], fout_accs[0][:])
                        nc.sync.dma_start(d_fout[:, 1024:2048], fout_accs[1][:])

                    # ====== final tail: up-projection + residual ======
                    with tc.tile_pool(name="psC", bufs=1, space="PSUM") as psC:
                        for q in range(4):
                            cs = slice(q * 512, (q + 1) * 512)
                            for a in range(2):
                                p = psC.tile([128, 512], F32, tag="up", bufs=4,
                                             name=f"p_up_{q}_{a}")
                                nc.tensor.matmul(
                                    p[:], t_wupt[:, a * 128:(a + 1) * 128],
                                    scaled[:, cs], start=True, stop=True)
                                out_s = sbM.tile([128, 512], F32, tag="tail",
                                                 bufs=4, name=f"out_s_{q}_{a}")
                                xlv = t_xl[:, q * 1024 + a * 512:
                                           q * 1024 + a * 512 + 512].bitcast(F32)
                                nc.vector.tensor_add(out_s[:], p[:], xlv)
                                nc.sync.dma_start(
                                    out[:, a * 2048 + q * 512:
                                        a * 2048 + (q + 1) * 512], out_s[:])

            sbP1_cm.__exit__(None, None, None)
            if debug:
                nc.sync.dma_start(d_fy2[:], fy2[:].bitcast(F32))
                nc.sync.dma_start(d_fx2[:], fx2[:].bitcast(F32))
                pass  # d_fself dump disabled (bf16)
                nc.sync.dma_start(d_h1s[:], h1s_aug[:].bitcast(F32))
                nc.sync.dma_start(d_scaled[:], scaled[:].bitcast(F32))

    nc.compile()
    return nc


def _prep_maps(x, y, W_self1, b_self1, W_self2, b_self2, W_x1, b_x1, W_x2,
               b_x2, W_y1, b_y1, W_y2, b_y2, W_up, b_up):
    f64 = np.float64

    def fold(W2, b1, b2):
        return (W2.astype(f64) @ b1.astype(f64) + b2.astype(f64)).astype(np.float32)

    ws2a = np.zeros((65, 66), np.float32)
    ws2a[64, 0] = 1.0
    ws2a[0:64, 1:65] = W_self2.T
    ws2a[64, 1:65] = fold(W_self2, b_self1, b_self2)
    bx2 = fold(W_x2, b_x1, b_x2).reshape(64, 1)
    _by2 = fold(W_y2, b_y1, b_y2).astype(np.float64)
    by2a = np.ascontiguousarray(
        np.stack([0.75 * _by2, 0.25 * _by2], axis=1).astype(np.float32))

    ws1t = np.ascontiguousarray(
        W_self1.T.reshape(2, 128, 64).transpose(1, 0, 2).reshape(128, 128))
    wx1t = np.ascontiguousarray(
        W_x1.T.reshape(2, 128, 64).transpose(1, 0, 2).reshape(128, 128))
    wy1t = np.ascontiguousarray(
        W_y1.T.reshape(4, 128, 64).transpose(1, 0, 2).reshape(128, 256))
    wx2t = np.ascontiguousarray(W_x2.T)
    wy2t = np.ascontiguousarray(W_y2.T)
    wupt = np.ascontiguousarray(
        np.concatenate([b_up.reshape(1, 256), W_up.T], axis=0))
    wp = np.zeros((128, 1093), np.float32)
    wp[:, 0:128] = ws1t
    wp[0:65, 128:194] = ws2a
    wp[:, 194:322] = wx1t
    wp[0:64, 322:386] = wx2t
    wp[0:64, 386:387] = bx2
    wp[:, 387:643] = wy1t
    wp[0:64, 643:707] = wy2t
    wp[0:64, 707:709] = by2a
    wp[0:65, 709:965] = wupt
    wp[0:1, 965:1093] = 1.0

    _ONES = np.ones((1, 4096), np.float32)
    maps = []
    for b in range(B):
        xf = x[b].reshape(CX, N)                                # [256, 4096]
        xs_h = np.ascontiguousarray(
            xf.reshape(2, 128, 8, 512).transpose(1, 2, 0, 3).reshape(128, 8192))
        yf = y[b].reshape(CY, NYC)
        yb_h = np.ascontiguousarray(
            yf.reshape(4, 128, 2, 512).transpose(1, 2, 0, 3).reshape(128, 4096))
        for half in range(2):
            xh = xf[:, half * NH:(half + 1) * NH]               # [256, 2048]
            xl_h = np.ascontiguousarray(
                xh.reshape(2, 128, 4, 512).transpose(1, 2, 0, 3).reshape(128, 4096))
            maps.append({
                "xs": xs_h, "xl": xl_h, "yb": yb_h,
                "wpack": wp, "ones": _ONES,
            })
    return maps


def _run(inputs, trace=False, trace_kwargs=None, debug=False):
    key = ("nc", debug)
    if key not in _CACHE:
        _CACHE[key] = _build(debug=debug)
    nc = _CACHE[key]
    maps = _prep_maps(**inputs)
    res = run_bass_kernel_spmd(nc, maps, list(range(8)), trace=trace,
                               **(trace_kwargs or {}))
    outs = np.empty((B, CX, H, W), np.float32)
    for b in range(B):
        for half in range(2):
            o = res.results[2 * b + half]["out"]                # [128, 4096]
            oh = o.reshape(128, 2, NH).transpose(1, 0, 2).reshape(CX, NH)
            outs[b, :, :, :].reshape(CX, N)[:, half * NH:(half + 1) * NH] = oh
    return outs, res


def kernel(**inputs):
    outs, _ = _run(inputs, trace=False)
    return outs

